# revision 1
# baseline (speedup 1.0000x reference)
"""Anisotropic Gaussian filter on 8 TRN2 NeuronCores (Bass/Tile).

Math per pixel p (global g = b*HW+p), window (i,j) in 7x7, r=3:
  dx = x[b,r,j,p] - x[b,i,j,p];  dy = x[b,i,r,p] - x[b,i,j,p]
  arg = -0.5*(sigx*dx)^2 - 0.5*(sigy*dy)^2 + theta*sigx*sigy*|dx*dy|
        - m*(0.5/sigr^2),   m = (i-r)^2 + (j-r)^2
  k = exp(arg);  out = sum_ij(k*x)/sum_ij(k)

Kernel dataflow (window-on-partition layout, 98 = 2 batches x 49;
T = 512-pixel quanta, processed as 32 super-tiles of 2 quanta, in a
3-stage software pipeline A/B/C with explicit skew + wait-ts pacing):
  - x stored f16 [98, HWC], SBUF-resident (8 tiles, upfront DMAs).
  - per-pixel coefficient rows (sigx^2, sigy^2, -2*theta*sigx*sigy)
    land as [98, RG*3T] SBUF f16 tiles via partition-broadcast DMAs
    (stride-0 source, one per batch half), prefetched 2 groups ahead,
    so all DVE multiplies run 2x f16 SBUF-to-SBUF.
  - A: PE computes raw diffs e12 = [dx|dy] from x directly (no input
    deps); one ACT Abs(scale=1/sqrt2) per quantum drains PSUM ->
    a12 = |[dx|dy]|/sqrt2.
  - B: DVE: pm = a1*a2, rmneg = pm*(-2 th sx sy), q12 = a12^2 *
    (sx^2|sy^2).  acc (PSUM, per quantum) = m*c (spa broadcast
    matmul) + I*rmneg + I*q1 + I*q2 via identity-matmul accumulation;
    k = Exp(acc, scale=-1) on ACT lands in ktw[:, k-slot].
  - C: w = k*x on the Pool engine into ktw[:, w-slot]; ONE reduction
    matmul per quantum (band-matrix lhsT, shifted column per tile)
    accumulates rows s/32+s, cols [sum k | sum kx] of a [64, 2T] PSUM
    bank over 32 tiles; per half: ACT drain, DVE recip/mult, 2 DMAs.
PSUM: e12 2x2 banks + acc 2x1 + red 2 = 8 banks, double-buffered.
"""

from contextlib import ExitStack

import numpy as np

B, KS, HW = 2, 7, 262144
NCORES = 8
HWC = HW // NCORES          # pixels per core per batch (32768)
T = 512                     # pixels per tile
NT = HWC // T               # tiles per core (64)
NH = NT // 32               # reduction halves (2)
P98 = 2 * KS * KS           # 98
R = KS // 2

_compiled = {}


def _build_weights():
    iy, ix = np.meshgrid(np.arange(KS), np.arange(KS), indexing="ij")
    m = ((iy - R) ** 2 + (ix - R) ** 2).reshape(-1).astype(np.float32)  # (49,)

    wsp = np.zeros((2, P98), np.float32)     # spa broadcast: acc += m*c
    wdx = np.zeros((P98, P98), np.float32)   # dx = x[center row] - x
    wdy = np.zeros((P98, P98), np.float32)
    id98 = np.eye(P98, dtype=np.float32)     # identity: PSUM accumulate adds
    for b in range(2):
        o = 49 * b
        wsp[b, o:o + 49] = m
        for w in range(49):
            i, j = divmod(w, KS)
            wdx[o + R * KS + j, o + w] += 1.0
            wdx[o + w, o + w] -= 1.0
            wdy[o + i * KS + R, o + w] += 1.0
            wdy[o + w, o + w] -= 1.0

    # band matrix for shifted reduction columns: slice [:, 31-s : 95-s]
    # gives ones at col s (batch0) / 32+s (batch1)
    bandw = np.zeros((P98, 95), np.float32)
    bandw[0:49, 31 + 0] = 1.0
    bandw[49:98, 31 + 32] = 1.0
    return wsp, wdx, wdy, id98, bandw


def _build_nc():
    import concourse.bacc as bacc
    import concourse.tile as tile
    from concourse import mybir

    f32 = mybir.dt.float32
    f16 = mybir.dt.float16
    i16 = mybir.dt.int16
    Alu = mybir.AluOpType
    Act = mybir.ActivationFunctionType

    nc = bacc.Bacc(enable_partition_id=False)
    x16d = nc.declare_dram_parameter("x16", [P98, HWC], f16, isOutput=False)
    repd = nc.declare_dram_parameter("rep", [2, NT, 3 * T], f16, isOutput=False)
    spvd = nc.declare_dram_parameter("spv", [2, HWC], f16, isOutput=False)
    wspd = nc.declare_dram_parameter("wsp", [2, P98], f16, isOutput=False)
    wdxd = nc.declare_dram_parameter("wdx", [P98, P98], f16, isOutput=False)
    wdyd = nc.declare_dram_parameter("wdy", [P98, P98], f16, isOutput=False)
    id98d = nc.declare_dram_parameter("id98", [P98, P98], f16, isOutput=False)
    bndd = nc.declare_dram_parameter("bandw", [P98, 95], f16, isOutput=False)
    out = nc.declare_dram_parameter("out", [2, HWC], f32, isOutput=True)

    XCH = 8                      # x upfront-load chunks
    CW = HWC // XCH

    with tile.TileContext(nc) as tc, ExitStack() as ctx:
        wpool = ctx.enter_context(tc.tile_pool(name="wpool", bufs=1))
        repp = ctx.enter_context(tc.tile_pool(name="repp", bufs=3))
        sqp = ctx.enter_context(tc.tile_pool(name="sqp", bufs=5))
        smp = ctx.enter_context(tc.tile_pool(name="smp", bufs=6))
        ktp = ctx.enter_context(tc.tile_pool(name="ktp", bufs=4))
        dns = ctx.enter_context(tc.tile_pool(name="dns", bufs=1))
        pS = ctx.enter_context(tc.tile_pool(name="pS", bufs=2, space="PSUM"))
        pacc = ctx.enter_context(tc.tile_pool(name="pacc", bufs=2, space="PSUM"))
        pred = ctx.enter_context(tc.tile_pool(name="pred", bufs=1, space="PSUM"))

        # one-time weights
        wspt = wpool.tile([2, P98], f16)
        nc.sync.dma_start(out=wspt[:], in_=wspd[:])
        wdxt = wpool.tile([P98, P98], f16)
        nc.sync.dma_start(out=wdxt[:], in_=wdxd[:])
        wdyt = wpool.tile([P98, P98], f16)
        nc.sync.dma_start(out=wdyt[:], in_=wdyd[:])
        id98t = wpool.tile([P98, P98], f16)
        nc.sync.dma_start(out=id98t[:], in_=id98d[:])
        bndt = wpool.tile([P98, 95], f16)
        nc.sync.dma_start(out=bndt[:], in_=bndd[:])

        # x resident in SBUF as 8 tiles (tile-granular deps: early compute
        # only waits on the first chunk)
        XW = HWC // XCH
        xtiles = []
        for xi in range(XCH):
            xt_ = wpool.tile([P98, XW], f16, name=f"xsb{xi}")
            nc.sync.dma_start(out=xt_[:], in_=x16d[:, xi * XW:(xi + 1) * XW])
            xtiles.append(xt_)

        def xs(t):
            j = (t * T) // XW
            o = t * T - j * XW
            return xtiles[j][:, o:o + T]

        # 3-stage software pipeline with explicit skew: each engine's
        # in-order stream alternates tiles so cross-engine waits are
        # resolved a full super-iteration ahead.
        #   A(t): rep DMA, uv, e1/e2 matmuls, |e| drain
        #   B(t): spa matmul, pm/rm/q12, acc identity matmuls, exp
        #   C(t): wt, reduction matmuls (+ half drain & store)
        reps, spvs, a12s, kts, reds = {}, {}, {}, {}, {}
        RG = 4                  # quanta (T-blocks) per grouped coefficient DMA

        def load_rep_group(g):
            # one partition-broadcast DMA covers RG quanta's coefficients
            grp = repp.tile([P98, RG * 3 * T], f16, tag="rep", name=f"repg{g}")
            for b in range(2):
                nc.sync.dma_start(
                    out=grp[49 * b:49 * (b + 1), :],
                    in_=repd[b, g * RG:(g + 1) * RG, :]
                        .rearrange("g f -> (g f)")[None, :]
                        .to_broadcast((49, RG * 3 * T)))
            spg = repp.tile([2, RG * T], f16, tag="spv", name=f"spvg{g}")
            nc.sync.dma_start(out=spg[:],
                              in_=spvd[:, g * RG * T:(g + 1) * RG * T])
            return grp, spg

        NG = NT // RG
        PF = 2                  # rep-group prefetch distance
        NTT = NT // 2           # super-tiles (2 quanta each)

        def stage_a(tt):
            t0 = 2 * tt
            if t0 == 0:
                for g in range(min(PF + 1, NG)):
                    reps[g], spvs[g] = load_rep_group(g)
            elif t0 % RG == 0 and t0 // RG + PF < NG:
                g = t0 // RG + PF
                reps[g], spvs[g] = load_rep_group(g)
            # raw diffs straight from resident x: PE A-stage has no deps
            a12 = sqp.tile([P98, 4 * T], f16, tag="a12")
            for q in range(2):
                qo = q * 2 * T
                xq = xs(t0 + q)
                e12 = pS.tile([P98, 2 * T], f32, tag="e12", name=f"e12_{tt}_{q}")
                nc.tensor.matmul(out=e12[:, 0:T], lhsT=wdxt[:],
                                 rhs=xq, start=True, stop=True)
                nc.tensor.matmul(out=e12[:, T:2 * T], lhsT=wdyt[:],
                                 rhs=xq, start=True, stop=True)
                # a12 = |[dx|dy]|/sqrt2
                nc.scalar.activation(out=a12[:, qo:qo + 2 * T], in_=e12[:],
                                     func=Act.Abs, scale=0.7071067811865476)
            a12s[tt] = a12

        def stage_b(tt):
            t0 = 2 * tt
            o = (t0 % RG) * 3 * T
            rep2 = reps[t0 // RG][:, o:o + 6 * T]
            spg = spvs[t0 // RG]
            if t0 % RG == RG - 2:
                del reps[t0 // RG], spvs[t0 // RG]
            a12 = a12s.pop(tt)
            # pm_raw = |dx dy|/2;  rm = pm_raw * (-2 theta sx sy)
            # (split per quantum: q1 half runs on the idle Pool engine)
            pm = smp.tile([P98, 2 * T], f16, tag="pm")
            for q in range(2):
                peng = nc.gpsimd if (PM_POOL and q == 1) else nc.vector
                peng.tensor_tensor(
                    out=pm[:, q * T:(q + 1) * T],
                    in0=a12[:, 2 * q * T:(2 * q + 1) * T],
                    in1=a12[:, (2 * q + 1) * T:(2 * q + 2) * T],
                    op=Alu.mult)
            rmneg = smp.tile([P98, 2 * T], f16, tag="rm")
            for q in range(2):
                nc.vector.tensor_tensor(
                    out=rmneg[:, q * T:(q + 1) * T],
                    in0=pm[:, q * T:(q + 1) * T],
                    in1=rep2[:, (3 * q + 2) * T:(3 * q + 3) * T],
                    op=Alu.mult)
            # q12 = (dx^2/2, dy^2/2) * (sx^2, sy^2), split per quantum;
            # the scale pass writes in place over the squares
            q12 = smp.tile([P98, 4 * T], f16, tag="q12")
            for q in range(2):
                qo = 2 * q * T
                nc.vector.tensor_tensor(out=q12[:, qo:qo + 2 * T],
                                        in0=a12[:, qo:qo + 2 * T],
                                        in1=a12[:, qo:qo + 2 * T],
                                        op=Alu.mult)
                nc.vector.tensor_tensor(
                    out=q12[:, qo:qo + 2 * T],
                    in0=q12[:, qo:qo + 2 * T],
                    in1=rep2[:, 3 * q * T:(3 * q + 2) * T],
                    op=Alu.mult)
            # ktw layout per quantum: [ k_q | k_q*x_q ] so one reduction
            # matmul per quantum covers both window sums
            ktw = ktp.tile([P98, 4 * T], f16, tag="ktw")
            for q in range(2):
                so = (t0 % RG + q) * T
                acc = pacc.tile([P98, T], f32, tag="acc",
                                name=f"acc_{tt}_{q}")
                nc.tensor.matmul(out=acc[:], lhsT=wspt[:],
                                 rhs=spg[:, so:so + T],
                                 start=True, stop=False)
                nc.tensor.matmul(out=acc[:], lhsT=id98t[:],
                                 rhs=rmneg[:, q * T:(q + 1) * T],
                                 start=False, stop=False)
                nc.tensor.matmul(out=acc[:], lhsT=id98t[:],
                                 rhs=q12[:, 2 * q * T:(2 * q + 1) * T],
                                 start=False, stop=False)
                nc.tensor.matmul(out=acc[:], lhsT=id98t[:],
                                 rhs=q12[:, (2 * q + 1) * T:(2 * q + 2) * T],
                                 start=False, stop=True)
                nc.scalar.activation(out=ktw[:, 2 * q * T:(2 * q + 1) * T],
                                     in_=acc[:], func=Act.Exp, scale=-1.0)
            kts[tt] = ktw

        def stage_c(tt):
            t0 = 2 * tt
            ktw = kts.pop(tt)
            for q in range(2):
                eng = nc.gpsimd if WT_POOL else nc.vector
                eng.tensor_tensor(
                    out=ktw[:, (2 * q + 1) * T:(2 * q + 2) * T],
                    in0=ktw[:, 2 * q * T:(2 * q + 1) * T],
                    in1=xs(t0 + q), op=Alu.mult)
            for q in range(2):
                t = t0 + q
                h, s = divmod(t, 32)
                if s == 0:
                    reds[h] = pred.tile([64, 2 * T], f32, tag="red",
                                        name=f"red{h}")
                red = reds[h]
                # rows s (b0), 32+s (b1): cols 0:T = sum k, T:2T = sum k*x
                # (two matmuls: a PSUM-bank write must stay within 2KB)
                nc.tensor.matmul(out=red[:, 0:T], lhsT=bndt[:, 31 - s:95 - s],
                                 rhs=ktw[:, 2 * q * T:(2 * q + 1) * T],
                                 start=(s == 0), stop=(s == 31),
                                 skip_group_check=True)
                nc.tensor.matmul(out=red[:, T:2 * T],
                                 lhsT=bndt[:, 31 - s:95 - s],
                                 rhs=ktw[:, (2 * q + 1) * T:(2 * q + 2) * T],
                                 start=(s == 0), stop=(s == 31),
                                 skip_group_check=True)
                if s == 31:
                    red = reds.pop(h)
                    dense = dns.tile([64, 2 * T], f32, tag="dense")
                    nc.scalar.copy(out=dense[:], in_=red[:])
                    rcp = dns.tile([64, T], f32, tag="rcp")
                    nc.vector.reciprocal(out=rcp[:], in_=dense[:, 0:T])
                    res = dns.tile([64, T], f32, tag="res")
                    nc.vector.tensor_tensor(out=res[:], in0=dense[:, T:2 * T],
                                            in1=rcp[:], op=Alu.mult)
                    for b in range(2):
                        ov = out[b, h * 32 * T:(h + 1) * 32 * T].rearrange(
                            "(t f) -> t f", f=T)
                        nc.sync.dma_start(out=ov,
                                          in_=res[32 * b:32 * b + 32, :])

        import os
        WT_POOL = os.environ.get("KWPOOL", "1") == "1"
        PM_POOL = os.environ.get("KPMPOOL", "1") == "1"
        DLT = float(os.environ.get("KD", "4.6"))  # us per super-iteration
        OFB = float(os.environ.get("KOB", "0.3"))
        OFC = float(os.environ.get("KOC", "0.6"))
        SORD = os.environ.get("KSORD", "abc")
        for u in range(NTT + 2):
            for st in SORD:
                if st == "a" and u < NTT:
                    with tc.tile_wait_until(u * DLT, enable=DLT > 0):
                        stage_a(u)
                if st == "b" and 1 <= u < NTT + 1:
                    with tc.tile_wait_until(u * DLT + OFB, enable=DLT > 0):
                        stage_b(u - 1)
                if st == "c" and u >= 2:
                    with tc.tile_wait_until(u * DLT + OFC, enable=DLT > 0):
                        stage_c(u - 2)

    if not nc.is_finalized():
        nc.finalize()
    return nc


def _run_pjrt(nc, in_maps):
    """Per-device single-core jits: this jax version's shard_map lowering
    emits multi-computation HLO that the bass_exec compile hook rejects,
    so dispatch one committed-args jit per NeuronCore instead (identical
    HLO -> the libneuronxla NEFF cache dedupes the 7 repeat compiles)."""
    import jax
    from jax import core as jcore
    from concourse import mybir
    from concourse.bass2jax import _bass_exec_p, install_neuronx_cc_hook

    install_neuronx_cc_hook()
    in_names, out_names, out_avals = [], [], []
    for alloc in nc.m.functions[0].allocations:
        if not isinstance(alloc, mybir.MemoryLocationSet):
            continue
        name = alloc.memorylocations[0].name
        if alloc.kind == "ExternalInput":
            in_names.append(name)
        elif alloc.kind == "ExternalOutput":
            out_avals.append(jcore.ShapedArray(
                tuple(alloc.tensor_shape), mybir.dt.np(alloc.dtype)))
            out_names.append(name)
    n_params = len(in_names)
    all_names = tuple(in_names) + tuple(out_names)
    donate = tuple(range(n_params, n_params + len(out_names)))

    def _body(*args):
        outs = _bass_exec_p.bind(
            *args, out_avals=tuple(out_avals), in_names=all_names,
            out_names=tuple(out_names), lowering_input_output_aliases=(),
            sim_require_finite=True, sim_require_nnan=True, nc=nc)
        return tuple(outs)

    fn = jax.jit(_body, donate_argnums=donate, keep_unused=True)
    devs = jax.devices()[:len(in_maps)]
    futs = []
    for c, m in enumerate(in_maps):
        args = [jax.device_put(np.ascontiguousarray(np.asarray(m[n])), devs[c])
                for n in in_names]
        args += [jax.device_put(np.zeros(a.shape, a.dtype), devs[c])
                 for a in out_avals]
        futs.append(fn(*args))
    jax.block_until_ready(futs)
    return [{name: np.asarray(f[i]) for i, name in enumerate(out_names)}
            for f in futs]


def prepare(x, sigx, sigy, theta, sigr):
    """Build (nc, in_maps) — shared by kernel() and test.py's profiler."""
    x = np.asarray(x, np.float32)
    sigx = np.asarray(sigx, np.float32)
    sigy = np.asarray(sigy, np.float32)
    theta = np.asarray(theta, np.float32)
    sigr = np.asarray(sigr, np.float32)

    if "nc" not in _compiled:
        _compiled["nc"] = _build_nc()
    nc = _compiled["nc"]

    wsp, wdx, wdy, id98, bandw = _build_weights()

    thneg = (-2.0 * theta).astype(np.float32)
    spv = (0.5 / (sigr.astype(np.float64) ** 2)).astype(np.float32)

    wsp16 = wsp.astype(np.float16)
    wdx16 = wdx.astype(np.float16)
    wdy16 = wdy.astype(np.float16)
    id16 = id98.astype(np.float16)
    bnd16 = bandw.astype(np.float16)

    in_maps = []
    for c in range(NCORES):
        rng = slice(c * HWC, (c + 1) * HWC)
        x_sh = np.ascontiguousarray(
            x[:, :, :, rng].reshape(P98, HWC)).astype(np.float16)
        rep_sh = np.empty((2, NT, 3, T), np.float32)
        spv_sh = np.empty((2, HWC), np.float32)
        for b in range(2):
            gr = slice(b * HW + c * HWC, b * HW + (c + 1) * HWC)
            rep_sh[b, :, 0, :] = (sigx[gr] ** 2).reshape(NT, T)
            rep_sh[b, :, 1, :] = (sigy[gr] ** 2).reshape(NT, T)
            rep_sh[b, :, 2, :] = (thneg[gr] * sigx[gr] * sigy[gr]
                                  ).reshape(NT, T)
            spv_sh[b] = spv[gr]
        in_maps.append({
            "x16": x_sh,
            "rep": rep_sh.reshape(2, NT, 3 * T).astype(np.float16),
            "spv": spv_sh.astype(np.float16),
            "wsp": wsp16, "wdx": wdx16, "wdy": wdy16,
            "id98": id16, "bandw": bnd16,
        })
    return nc, in_maps


def kernel(x, sigx, sigy, theta, sigr):
    nc, in_maps = prepare(x, sigx, sigy, theta, sigr)
    results = _run_pjrt(nc, in_maps)
    outs = [results[c]["out"] for c in range(NCORES)]
    return np.concatenate(outs, axis=1).astype(np.float32)

